# revision 22
# baseline (speedup 1.0000x reference)
"""Bass/Trainium2 kernel for nn_EnhancedMultiHeadAttention (sparse_attention).

out[b,h,i,j] = softmax_j( (q_bh i . k_bh j) * sc + relbias[b,i,j] + mask_term[b,i,j] )
  q = query @ Wq.T + bq   (sc = 1/sqrt(64) folded into Wq/bq on host)
  relbias[b,i,j] = (mean_h q[b,h,i,:]) . rel_k_table[clip(j-i,-128,128)+128, :] * sc
  mask_term = 0 where mask==1 else -30000

Sharding: 8 cores = 4 batches x 2 head-halves (8 heads per core).

v5 design notes (engine-balanced):
  - Output fp16 on device (halves the dominant output DMA); host casts.
  - comb[m] = (mask*30000 - 30000) + relbias_band, fp16, shared by all 8
    heads; mask term on gpsimd, band via the DRAM skew trick.
  - Row-tile classes:
      m in ID_MS: comb into PSUM via identity matmuls (PE); per-head exp
        with ACT accumulator row sums (sums ~free on ACT).
      m in E_MS : plain exp, then one 1x DVE scalar_tensor_tensor
        computing e*exp(comb) AND the row sum in a single pass.
    This splits the comb/sum work between PE and DVE under the ~64us
    ACT exp floor.
  - PSUM: per-head score tiles [128,1024] (pool bufs=3, 6 banks) +
    separate projection pool (bufs=1, 2 banks) so projection matmuls
    never stall the score/exp pipeline; projections emitted in small
    chunks between pairs.
  - Prologue: wpad writes on the scalar HWDGE queue (not behind input
    loads), skew reads on sync interleaved with the later weight
    slices; wppool/bpool fully buffered (bufs=8); prologue bias evacs
    on ACT (idle before the first exp).
  - DVE comb band chains + E=exp(comb) interleaved into the t=0
    emission to avoid FIFO head-of-line blocking.
"""

import numpy as np

B, S, D, H = 4, 1024, 1024, 16
DK = 64          # head dim
MAXREL = 128
NREL = 2 * MAXREL + 1          # 257
WPADW = 2 * MAXREL + NREL - 2  # 511 = 127 + 257 + 127
NRELP = 260     # rel matmul free dim padded for fp32r ISA restrictions
HPC = 8          # heads per core
DHALF = 512      # projected dims per core
NCORES = 8
PT = 128         # partition tile
NT = S // PT     # 8 row tiles

MASKV = 30000.0  # fp16-safe large negative bias for masked entries

ID_MS = (0, 1, 2, 3)     # identity-matmul comb + ACT accumulator sums
E_MS = (4, 5, 6, 7)      # e*exp(comb) + row sums via one DVE stt pass

_CACHE = {}


def _build():
    from contextlib import ExitStack

    import concourse.bass as bass
    import concourse.mybir as mybir
    import concourse.tile as tile
    from concourse import bacc
    from concourse.tile import add_dep_helper

    F32 = mybir.dt.float32
    F16 = mybir.dt.float16
    I8 = mybir.dt.int8
    AF = mybir.ActivationFunctionType
    ALU = mybir.AluOpType

    nc = bacc.Bacc("TRN2", target_bir_lowering=False, debug=False)

    xT = nc.dram_tensor("xT", [D, S], F16, kind="ExternalInput")
    kTx = nc.dram_tensor("kTx", [D, S], F16, kind="ExternalInput")
    maskb = nc.dram_tensor("maskb", [S, S], I8, kind="ExternalInput")
    wqT = nc.dram_tensor("wqT", [D, DHALF], F16, kind="ExternalInput")
    wkT = nc.dram_tensor("wkT", [D, DHALF], F16, kind="ExternalInput")
    bq4 = nc.dram_tensor("bq4", [PT, 4], F32, kind="ExternalInput")
    bk4 = nc.dram_tensor("bk4", [PT, 4], F32, kind="ExternalInput")
    wmT = nc.dram_tensor("wmT", [D, DK], F16, kind="ExternalInput")
    bm1 = nc.dram_tensor("bm1", [DK, 1], F32, kind="ExternalInput")
    tT = nc.dram_tensor("tT", [DK, NRELP], F16, kind="ExternalInput")
    out_d = nc.dram_tensor("out", [HPC, S, S], F16, kind="ExternalOutput")
    wpad_d = nc.dram_tensor("wpad_scratch", [S, WPADW], F16)
    ident_d = nc.inline_tensor(np.eye(PT, dtype=np.float16), "ident")

    with tile.TileContext(nc) as tc, ExitStack() as ctx:
        persist = ctx.enter_context(tc.tile_pool(name="persist", bufs=1))
        bpool = ctx.enter_context(tc.tile_pool(name="bpool", bufs=1))
        wppool = ctx.enter_context(tc.tile_pool(name="wppool", bufs=8))
        epool = ctx.enter_context(tc.tile_pool(name="epool", bufs=6))
        mpool = ctx.enter_context(tc.tile_pool(name="mpool", bufs=4))
        opool = ctx.enter_context(tc.tile_pool(name="opool", bufs=4))
        spool = ctx.enter_context(tc.tile_pool(name="spool", bufs=8))
        ph = ctx.enter_context(tc.tile_pool(name="ph", bufs=3, space="PSUM"))
        pj = ctx.enter_context(tc.tile_pool(name="pj", bufs=1, space="PSUM"))

        # trigger the ACT exp table load right away (off the critical path)
        tlw_sb = persist.tile([PT, 4], F32, tag="tlw")
        nc.vector.memset(tlw_sb[:], 0.0)
        nc.scalar.activation(tlw_sb[:], tlw_sb[:], AF.Exp, bias=0.0, scale=1.0)

        # ---- small constants ----
        id_sb = persist.tile([PT, PT], F16, tag="ident")
        nc.sync.dma_start(id_sb[:], ident_d[:])
        bq_sb = persist.tile([PT, 4], F32, tag="bq")
        nc.sync.dma_start(bq_sb[:], bq4[:])
        bk_sb = persist.tile([PT, 4], F32, tag="bk")
        nc.sync.dma_start(bk_sb[:], bk4[:])
        bm_sb = persist.tile([DK, 1], F32, tag="bm")
        nc.sync.dma_start(bm_sb[:], bm1[:])
        tT_sb = persist.tile([DK, NRELP], F16, tag="tT")
        nc.sync.dma_start(tT_sb[:], tT[:])
        ones_sb = persist.tile([PT, MAXREL - 1], F32, tag="ones")
        nc.vector.memset(ones_sb[:], 1.0)

        # ---- PE warmup while first DMAs land ----
        warm_sb = persist.tile([PT, DHALF], F16, tag="warm")
        nc.vector.memset(warm_sb[:], 0.0)
        wps = ph.tile([PT, S], F32, tag="ph", name="warmps")
        for i in range(6):
            nc.tensor.matmul(wps[:, 0:DHALF], id_sb[:], warm_sb[:],
                             start=True, stop=True)

        # ---- bulk input loads (sync queue, critical-path order) ----
        def load_chunked(name, dram, width, dt_, parts, eng):
            t = persist.tile([PT, NT * width], dt_, tag=name, name=name)
            cpp = NT // parts
            for pi in range(parts):
                srcap = bass.AP(dram, pi * cpp * PT * width,
                                [[width, PT], [PT * width, cpp], [1, width]])
                eng.dma_start(
                    t[:, pi * cpp * width:(pi + 1) * cpp * width]
                    .rearrange("p (c s) -> p c s", s=width), srcap)
            return t

        wm_all = load_chunked("wm_all", wmT, DK, F16, 1, nc.sync)
        x_all = load_chunked("x_all", xT, S, F16, 4, nc.sync)
        wm_sb = [wm_all[:, kc * DK:(kc + 1) * DK] for kc in range(NT)]
        x_tiles = [x_all[:, kc * S:(kc + 1) * S] for kc in range(NT)]

        wqt_sb = [persist.tile([PT, NT * PT], F16, tag=f"wqt{t}",
                               name=f"wqt{t}") for t in range(4)]
        wkt_sb = [persist.tile([PT, NT * PT], F16, tag=f"wkt{t}",
                               name=f"wkt{t}") for t in range(4)]

        def load_wslice(dst, dram, t):
            srcap = bass.AP(dram, t * PT,
                            [[DHALF, PT], [PT * DHALF, NT], [1, PT]])
            nc.sync.dma_start(
                dst[:].rearrange("p (c s) -> p c s", s=PT), srcap)

        k_all = persist.tile([PT, NT * S], F16, tag="k_all", name="k_all")
        load_wslice(wqt_sb[0], wqT, 0)
        for pi in range(4):
            srcap = bass.AP(kTx, pi * 2 * PT * S,
                            [[S, PT], [PT * S, 2], [1, S]])
            nc.sync.dma_start(
                k_all[:, pi * 2 * S:(pi + 1) * 2 * S]
                .rearrange("p (c s) -> p c s", s=S), srcap)
        load_wslice(wkt_sb[0], wkT, 0)
        k_tiles = [k_all[:, kc * S:(kc + 1) * S] for kc in range(NT)]

        # mask chunks on sync AFTER the critical x/k/weight loads so they
        # don't steal DMA bandwidth from the pipeline-start path
        mask_all = persist.tile([PT, NT * S], I8, tag="mask_all")
        for m in range(NT):
            nc.sync.dma_start(
                mask_all[:, m * S:(m + 1) * S],
                bass.AP(maskb, m * PT * S, [[S, PT], [1, S]]))
        # mask terms into comb tiles (gpsimd compute, early)
        comb_sb = [persist.tile([PT, S], F16, tag=f"comb{m}", name=f"comb{m}")
                   for m in range(NT)]
        for m in range(NT):
            nc.gpsimd.tensor_scalar(comb_sb[m][:],
                                    mask_all[:, m * S:(m + 1) * S],
                                    MASKV, -MASKV, ALU.mult, ALU.add)

        # ---- qm + t=0 q projection, interleaved per x chunk ----
        qmT_sb = persist.tile([DK, S], F16, tag="qmT")
        qT_sb = [persist.tile([PT, S], F16, tag=f"qT{i}", name=f"qT{i}")
                 for i in range(4)]
        kT_sb = [persist.tile([PT, S], F16, tag=f"kT{i}", name=f"kT{i}")
                 for i in range(4)]

        qmps = ph.tile([PT, S], F32, tag="ph", name="qmps")
        pq0 = pj.tile([PT, S], F32, tag="pj", name="projq0")
        for kc in range(NT):
            for nh in range(2):
                nhs = slice(nh * DHALF, (nh + 1) * DHALF)
                nc.tensor.matmul(qmps[0:DK, nhs], wm_sb[kc][:],
                                 x_tiles[kc][:, nhs],
                                 start=(kc == 0), stop=(kc == NT - 1))
            for nh in range(2):
                nhs = slice(nh * DHALF, (nh + 1) * DHALF)
                nc.tensor.matmul(pq0[:, nhs],
                                 wqt_sb[0][:, kc * PT:(kc + 1) * PT],
                                 x_tiles[kc][:, nhs],
                                 start=(kc == 0), stop=(kc == NT - 1))
        # prologue evacs on ACT (idle until the first exp)
        nc.scalar.activation(qmT_sb[:], qmps[0:DK, 0:S], AF.Identity,
                             bias=bm_sb[:], scale=1.0)
        nc.scalar.activation(qT_sb[0][:], pq0[:], AF.Identity,
                             bias=bq_sb[:, 0:1], scale=1.0)

        # ---- rel-bias W matmuls + wpad builds; wpad writes on the
        # scalar HWDGE queue, skew reads collected for the sync queue ----
        band_info = [(max(0, PT * (m - 1)), min(S, PT * (m + 2)))
                     for m in range(NT)]
        w0_sb = [persist.tile([PT, 1], F32, tag=f"w0_{m}", name=f"w0_{m}")
                 for m in range(NT)]
        w256_sb = [persist.tile([PT, 1], F32, tag=f"w256_{m}",
                                name=f"w256_{m}") for m in range(NT)]
        band_sb = [None] * NT
        wr_ins = [None] * NT

        def w_chain(m):
            jlo, jhi = band_info[m]
            ps = pj.tile([PT, S], F32, tag="pj", name=f"wps{m}")
            nc.tensor.matmul(ps[:, 0:NRELP], qmT_sb[:, m * PT:(m + 1) * PT],
                             tT_sb[:], start=True, stop=True)
            wp = wppool.tile([PT, WPADW], F16, tag="wpad", name=f"wpad{m}")
            nc.vector.tensor_copy(wp[:, MAXREL - 1:MAXREL - 1 + NREL],
                                  ps[:, 0:NREL])
            nc.vector.tensor_copy(w0_sb[m][:], ps[:, 0:1])
            nc.vector.tensor_copy(w256_sb[m][:], ps[:, NREL - 1:NREL])
            nc.vector.tensor_scalar_mul(wp[:, 0:MAXREL - 1], ones_sb[:],
                                        w0_sb[m][:])
            nc.vector.tensor_scalar_mul(wp[:, MAXREL - 1 + NREL:WPADW],
                                        ones_sb[:], w256_sb[m][:])
            wr_ins[m] = nc.scalar.dma_start(wpad_d[m * PT:(m + 1) * PT, :],
                                            wp[:])
            band_sb[m] = bpool.tile([PT, jhi - jlo], F16, tag=f"band{m}",
                                    name=f"band{m}")

        def band_read(m):
            jlo, jhi = band_info[m]
            srcap = bass.AP(wpad_d, PT * (WPADW - 1) * m + jlo + (WPADW // 2),
                            [[WPADW - 1, PT], [1, jhi - jlo]])
            ri = nc.sync.dma_start(band_sb[m][:], srcap)
            add_dep_helper(ri.ins, wr_ins[m].ins, reason="wpad DRAM RAW")

        for m in range(4):
            w_chain(m)
        for m in range(4):
            band_read(m)

        # ---- t=0 k projection (k chunks land about now) ----
        pk0 = pj.tile([PT, S], F32, tag="pj", name="projk0")
        for kc in range(NT):
            for nh in range(2):
                nhs = slice(nh * DHALF, (nh + 1) * DHALF)
                nc.tensor.matmul(pk0[:, nhs],
                                 wkt_sb[0][:, kc * PT:(kc + 1) * PT],
                                 k_tiles[kc][:, nhs],
                                 start=(kc == 0), stop=(kc == NT - 1))
        nc.scalar.activation(kT_sb[0][:], pk0[:], AF.Identity,
                             bias=bk_sb[:, 0:1], scale=1.0)

        for m in range(4, NT):
            w_chain(m)
        for m in range(4, NT):
            band_read(m)
        # remaining per-t weight slices (after the band reads on sync)
        for t in range(1, 4):
            load_wslice(wqt_sb[t], wqT, t)
            load_wslice(wkt_sb[t], wkT, t)

        # ---- comb band chains (DVE); E = exp(comb) for E_MS (ACT) ----
        E_sb = {m: persist.tile([PT, S], F16, tag=f"E{m}", name=f"E{m}")
                for m in E_MS}

        def comb_chain(m):
            jlo, jhi = band_info[m]
            cb = comb_sb[m]
            nc.vector.tensor_add(cb[:, jlo:jhi], cb[:, jlo:jhi],
                                 band_sb[m][:])
            if jlo > 0:
                nc.vector.tensor_scalar_add(cb[:, 0:jlo], cb[:, 0:jlo],
                                            w0_sb[m][:])
            if jhi < S:
                nc.vector.tensor_scalar_add(cb[:, jhi:S], cb[:, jhi:S],
                                            w256_sb[m][:])

        comb_chain(0)
        comb_chain(1)

        # ---- projection op chunks for t>=1 ----
        def proj_ops(t, w_t, x_t, dst, bias_sb, nm):
            ps = [None]
            ops = []
            for kc in range(NT):
                for nh in range(2):
                    nhs = slice(nh * DHALF, (nh + 1) * DHALF)
                    def mm(kc=kc, nhs=nhs, first=(kc == 0 and nhs.start == 0)):
                        if first:
                            ps[0] = pj.tile([PT, S], F32, tag="pj",
                                            name=f"proj{nm}{t}")
                        nc.tensor.matmul(ps[0][:, nhs],
                                         w_t[:, kc * PT:(kc + 1) * PT],
                                         x_t[kc][:, nhs],
                                         start=(kc == 0), stop=(kc == NT - 1))
                    ops.append(mm)

            def evac():
                nc.vector.tensor_scalar_add(dst[:], ps[0][:],
                                            bias_sb[:, t:t + 1])
            ops.append(evac)
            return ops

        # ---- main loop: 4 head pairs x 8 row tiles ----
        for t in range(4):
            pending = []
            if t < 3:
                pending = (proj_ops(t + 1, wqt_sb[t + 1], x_tiles,
                                    qT_sb[t + 1], bq_sb, "q")
                           + proj_ops(t + 1, wkt_sb[t + 1], k_tiles,
                                      kT_sb[t + 1], bk_sb, "k"))
            for m in range(NT):
                mb = slice(m * PT, (m + 1) * PT)
                is_id = m in ID_MS
                psA = ph.tile([PT, S], F32, tag="ph", name=f"psA_{t}_{m}")
                psB = ph.tile([PT, S], F32, tag="ph", name=f"psB_{t}_{m}")
                for hb, psx, tp in ((0, psA, (0, 0)), (1, psB, (64, 0))):
                    qsl = qT_sb[t][hb * DK:(hb + 1) * DK, mb]
                    for nh in range(2):
                        nhs = slice(nh * DHALF, (nh + 1) * DHALF)
                        nc.tensor.matmul(psx[:, nhs], qsl,
                                         kT_sb[t][hb * DK:(hb + 1) * DK, nhs],
                                         start=True, stop=not is_id,
                                         tile_position=tp)
                if is_id:
                    for psx in (psA, psB):
                        for nh in range(2):
                            nhs = slice(nh * DHALF, (nh + 1) * DHALF)
                            nc.tensor.matmul(psx[:, nhs], id_sb[:],
                                             comb_sb[m][:, nhs],
                                             start=False, stop=True)

                # deferred DVE comb chains + E exps, spread over t=0
                if t == 0 and m < 3:
                    comb_chain(2 * m + 2)
                    comb_chain(2 * m + 3)
                if t == 0 and 2 <= m < 6:
                    me = m + 2
                    nc.scalar.activation(E_sb[me][:], comb_sb[me][:], AF.Exp,
                                         bias=0.0, scale=1.0)

                e2 = epool.tile([PT, 2 * S], F16, tag="e", name=f"e{t}_{m}")
                S2 = spool.tile([PT, 2], F32, tag="s", name=f"s{t}_{m}")
                r2 = spool.tile([PT, 2], F32, tag="r", name=f"r{t}_{m}")
                o2 = opool.tile([PT, 2 * S], F16, tag="o", name=f"o{t}_{m}")
                srcs = []
                for hi, psx in ((0, psA), (1, psB)):
                    e = e2[:, hi * S:(hi + 1) * S]
                    sa = S2[:, hi:hi + 1]
                    if is_id:
                        nc.scalar.activation(e, psx[:], AF.Exp, bias=0.0,
                                             scale=1.0, accum_out=sa)
                        srcs.append(e)
                    else:
                        nc.scalar.activation(e, psx[:], AF.Exp, bias=0.0,
                                             scale=1.0)
                        mx = mpool.tile([PT, S], F16, tag="m",
                                        name=f"m{t}{hi}_{m}")
                        nc.vector.scalar_tensor_tensor(
                            mx[:], e, 1.0, E_sb[m][:], ALU.mult, ALU.mult,
                            accum_out=sa)
                        srcs.append(mx[:])
                nc.vector.reciprocal(r2[:], S2[:])
                for hi in range(2):
                    nc.vector.tensor_scalar_mul(o2[:, hi * S:(hi + 1) * S],
                                                srcs[hi], r2[:, hi:hi + 1])
                dst = bass.AP(out_d, (2 * t) * S * S + m * PT * S,
                              [[S, PT], [S * S, 2], [1, S]])
                nc.sync.dma_start(dst,
                                  o2[:].rearrange("p (h s) -> p h s", s=S))

                # feed pending projection matmuls to the PE in small chunks
                if pending and m >= 1:
                    nslots = NT - m
                    take = (len(pending) + nslots - 1) // nslots
                    for _ in range(min(take, 6)):
                        if pending:
                            pending.pop(0)()
            while pending:
                pending.pop(0)()

    nc.compile()
    return nc


def _get_nc():
    if "nc" not in _CACHE:
        _CACHE["nc"] = _build()
    return _CACHE["nc"]


def _prep_inputs(query, key, mask, Wq, bq, Wk, bk, rel_k_table):
    """Host-side sharding prep -> 8 per-core input dicts."""
    sc = 1.0 / np.sqrt(np.float32(DK))
    query = np.asarray(query, dtype=np.float32)
    key = np.asarray(key, dtype=np.float32)
    mask8 = np.ascontiguousarray(np.asarray(mask).astype(np.int8))
    Wq = np.asarray(Wq, dtype=np.float32)
    bq = np.asarray(bq, dtype=np.float32)
    Wk = np.asarray(Wk, dtype=np.float32)
    bk = np.asarray(bk, dtype=np.float32)
    T = np.asarray(rel_k_table, dtype=np.float32)

    WqTs = np.ascontiguousarray((Wq * sc).T)       # [D, D]
    WkT = np.ascontiguousarray(Wk.T)               # [D, D]
    bqs = bq * sc
    Wm16 = np.ascontiguousarray(((Wq.reshape(H, DK, D).mean(0) * sc).T).astype(np.float16))
    bm = (bq.reshape(H, DK).mean(0) * sc).reshape(DK, 1).astype(np.float32)
    tTc16 = np.zeros((DK, NRELP), np.float16)
    tTc16[:, :NREL] = T.T.astype(np.float16)       # [64, 260] zero-padded

    xT = [np.ascontiguousarray(query[b].T.astype(np.float16)) for b in range(B)]
    kT = [np.ascontiguousarray(key[b].T.astype(np.float16)) for b in range(B)]

    in_maps = []
    for c in range(NCORES):
        b, hh = divmod(c, 2)
        cols = slice(hh * DHALF, (hh + 1) * DHALF)
        in_maps.append(dict(
            xT=xT[b], kTx=kT[b], maskb=mask8[b],
            wqT=np.ascontiguousarray(WqTs[:, cols].astype(np.float16)),
            wkT=np.ascontiguousarray(WkT[:, cols].astype(np.float16)),
            bq4=np.ascontiguousarray(bqs[cols].reshape(4, PT).T),
            bk4=np.ascontiguousarray(bk[cols].reshape(4, PT).T),
            wmT=Wm16, bm1=bm, tT=tTc16,
        ))
    return in_maps


def run(inputs: dict, trace: bool = False):
    from concourse.bass_utils import run_bass_kernel_spmd

    nc = _get_nc()
    in_maps = _prep_inputs(**inputs)
    res = run_bass_kernel_spmd(nc, in_maps, core_ids=list(range(NCORES)),
                               trace=trace)
    out = np.empty((B, H, S, S), dtype=np.float32)
    for c in range(NCORES):
        b, hh = divmod(c, 2)
        out[b, hh * HPC:(hh + 1) * HPC] = res.results[c]["out"].astype(np.float32)
    return out, res


def kernel(**inputs) -> np.ndarray:
    out, _ = run(inputs)
    return out


# revision 28
# speedup vs baseline: 1.0414x; 1.0414x over previous
"""Bass/Trainium2 kernel for nn_EnhancedMultiHeadAttention (sparse_attention).

out[b,h,i,j] = softmax_j( (q_bh i . k_bh j) * sc + relbias[b,i,j] + mask_term[b,i,j] )
  q = query @ Wq.T + bq   (sc = 1/sqrt(64) folded into Wq/bq on host)
  relbias[b,i,j] = (mean_h q[b,h,i,:]) . rel_k_table[clip(j-i,-128,128)+128, :] * sc
  mask_term = 0 where mask==1 else -30000

Sharding: 8 cores = 4 batches x 2 head-halves (8 heads per core).

v5 design notes (engine-balanced):
  - Output fp16 on device (halves the dominant output DMA); host casts.
  - comb[m] = (mask*30000 - 30000) + relbias_band, fp16, shared by all 8
    heads; mask term on gpsimd, band via the DRAM skew trick.
  - Row-tile classes:
      m in ID_MS: comb into PSUM via identity matmuls (PE); per-head exp
        with ACT accumulator row sums (sums ~free on ACT).
      m in E_MS : plain exp, then one 1x DVE scalar_tensor_tensor
        computing e*exp(comb) AND the row sum in a single pass.
    This splits the comb/sum work between PE and DVE under the ~64us
    ACT exp floor.
  - PSUM: per-head score tiles [128,1024] (pool bufs=3, 6 banks) +
    separate projection pool (bufs=1, 2 banks) so projection matmuls
    never stall the score/exp pipeline; projections emitted in small
    chunks between pairs.
  - Prologue: wpad writes on the scalar HWDGE queue (not behind input
    loads), skew reads on sync interleaved with the later weight
    slices; wppool/bpool fully buffered (bufs=8); prologue bias evacs
    on ACT (idle before the first exp).
  - DVE comb band chains + E=exp(comb) interleaved into the t=0
    emission to avoid FIFO head-of-line blocking.
"""

import numpy as np

B, S, D, H = 4, 1024, 1024, 16
DK = 64          # head dim
MAXREL = 128
NREL = 2 * MAXREL + 1          # 257
WPADW = 2 * MAXREL + NREL - 2  # 511 = 127 + 257 + 127
NRELP = 260     # rel matmul free dim padded for fp32r ISA restrictions
HPC = 8          # heads per core
DHALF = 512      # projected dims per core
NCORES = 8
PT = 128         # partition tile
NT = S // PT     # 8 row tiles

MASKV = 30000.0  # fp16-safe large negative bias for masked entries

ID_MS = (0, 1, 2, 3)     # identity-matmul comb + ACT accumulator sums
E_MS = (4, 5, 6, 7)      # e*exp(comb) + row sums via one DVE stt pass

_CACHE = {}


def _build():
    from contextlib import ExitStack

    import concourse.bass as bass
    import concourse.mybir as mybir
    import concourse.tile as tile
    from concourse import bacc
    from concourse.tile import add_dep_helper

    F32 = mybir.dt.float32
    F16 = mybir.dt.float16
    I8 = mybir.dt.int8
    AF = mybir.ActivationFunctionType
    ALU = mybir.AluOpType

    nc = bacc.Bacc("TRN2", target_bir_lowering=False, debug=False)

    xT = nc.dram_tensor("xT", [D, S], F16, kind="ExternalInput")
    kTx = nc.dram_tensor("kTx", [D, S], F16, kind="ExternalInput")
    maskb = nc.dram_tensor("maskb", [S, S], I8, kind="ExternalInput")
    wqT = nc.dram_tensor("wqT", [D, DHALF], F16, kind="ExternalInput")
    wkT = nc.dram_tensor("wkT", [D, DHALF], F16, kind="ExternalInput")
    bq4 = nc.dram_tensor("bq4", [PT, 4], F32, kind="ExternalInput")
    bk4 = nc.dram_tensor("bk4", [PT, 4], F32, kind="ExternalInput")
    wmT = nc.dram_tensor("wmT", [D, DK], F16, kind="ExternalInput")
    bm1 = nc.dram_tensor("bm1", [DK, 1], F32, kind="ExternalInput")
    tT = nc.dram_tensor("tT", [DK, NRELP], F16, kind="ExternalInput")
    out_d = nc.dram_tensor("out", [HPC, S, S], F16, kind="ExternalOutput")
    wpad_d = nc.dram_tensor("wpad_scratch", [S, WPADW], F16)
    ident_d = nc.inline_tensor(np.eye(PT, dtype=np.float16), "ident")

    with tile.TileContext(nc) as tc, ExitStack() as ctx:
        persist = ctx.enter_context(tc.tile_pool(name="persist", bufs=1))
        bpool = ctx.enter_context(tc.tile_pool(name="bpool", bufs=1))
        wppool = ctx.enter_context(tc.tile_pool(name="wppool", bufs=8))
        epool = ctx.enter_context(tc.tile_pool(name="epool", bufs=6))
        mpool = ctx.enter_context(tc.tile_pool(name="mpool", bufs=4))
        opool = ctx.enter_context(tc.tile_pool(name="opool", bufs=4))
        spool = ctx.enter_context(tc.tile_pool(name="spool", bufs=8))
        ph = ctx.enter_context(tc.tile_pool(name="ph", bufs=3, space="PSUM"))
        pj = ctx.enter_context(tc.tile_pool(name="pj", bufs=1, space="PSUM"))

        # trigger the ACT exp table load right away (off the critical path)
        tlw_sb = persist.tile([PT, 4], F32, tag="tlw")
        nc.vector.memset(tlw_sb[:], 0.0)
        nc.scalar.activation(tlw_sb[:], tlw_sb[:], AF.Exp, bias=0.0, scale=1.0)

        # ---- small constants ----
        id_sb = persist.tile([PT, PT], F16, tag="ident")
        nc.sync.dma_start(id_sb[:], ident_d[:])
        bq_sb = persist.tile([PT, 4], F32, tag="bq")
        nc.sync.dma_start(bq_sb[:], bq4[:])
        bk_sb = persist.tile([PT, 4], F32, tag="bk")
        nc.sync.dma_start(bk_sb[:], bk4[:])
        bm_sb = persist.tile([DK, 1], F32, tag="bm")
        nc.sync.dma_start(bm_sb[:], bm1[:])
        tT_sb = persist.tile([DK, NRELP], F16, tag="tT")
        nc.sync.dma_start(tT_sb[:], tT[:])
        ones_sb = persist.tile([PT, MAXREL - 1], F32, tag="ones")
        nc.vector.memset(ones_sb[:], 1.0)

        # ---- PE warmup while first DMAs land ----
        warm_sb = persist.tile([PT, DHALF], F16, tag="warm")
        nc.vector.memset(warm_sb[:], 0.0)
        wps = ph.tile([PT, S], F32, tag="ph", name="warmps")
        for i in range(6):
            nc.tensor.matmul(wps[:, 0:DHALF], id_sb[:], warm_sb[:],
                             start=True, stop=True)

        # ---- bulk input loads (sync queue, critical-path order) ----
        def load_chunked(name, dram, width, dt_, parts, eng):
            t = persist.tile([PT, NT * width], dt_, tag=name, name=name)
            cpp = NT // parts
            for pi in range(parts):
                srcap = bass.AP(dram, pi * cpp * PT * width,
                                [[width, PT], [PT * width, cpp], [1, width]])
                eng.dma_start(
                    t[:, pi * cpp * width:(pi + 1) * cpp * width]
                    .rearrange("p (c s) -> p c s", s=width), srcap)
            return t

        wm_all = load_chunked("wm_all", wmT, DK, F16, 1, nc.sync)
        x_all = load_chunked("x_all", xT, S, F16, 4, nc.sync)
        wm_sb = [wm_all[:, kc * DK:(kc + 1) * DK] for kc in range(NT)]
        x_tiles = [x_all[:, kc * S:(kc + 1) * S] for kc in range(NT)]

        wq_all = load_chunked("wq_all", wqT, DHALF, F16, 1, nc.sync)
        k_all = persist.tile([PT, NT * S], F16, tag="k_all", name="k_all")
        for pi in range(4):
            srcap = bass.AP(kTx, pi * 2 * PT * S,
                            [[S, PT], [PT * S, 2], [1, S]])
            nc.sync.dma_start(
                k_all[:, pi * 2 * S:(pi + 1) * 2 * S]
                .rearrange("p (c s) -> p c s", s=S), srcap)
        wk_all = load_chunked("wk_all", wkT, DHALF, F16, 1, nc.sync)
        k_tiles = [k_all[:, kc * S:(kc + 1) * S] for kc in range(NT)]
        wq_tiles = [wq_all[:, kc * DHALF:(kc + 1) * DHALF] for kc in range(NT)]
        wk_tiles = [wk_all[:, kc * DHALF:(kc + 1) * DHALF] for kc in range(NT)]

        # mask chunks on sync AFTER the critical x/k/weight loads so they
        # don't steal DMA bandwidth from the pipeline-start path (first two
        # chunks early for the comb chains, rest after the band reads)
        mask_all = persist.tile([PT, NT * S], I8, tag="mask_all")
        comb_sb = [persist.tile([PT, S], F16, tag=f"comb{m}", name=f"comb{m}")
                   for m in range(NT)]

        def mask_load(m):
            nc.sync.dma_start(
                mask_all[:, m * S:(m + 1) * S],
                bass.AP(maskb, m * PT * S, [[S, PT], [1, S]]))

        def mask_term(m):
            nc.gpsimd.tensor_scalar(comb_sb[m][:],
                                    mask_all[:, m * S:(m + 1) * S],
                                    MASKV, -MASKV, ALU.mult, ALU.add)

        # ---- qm + t=0 q projection, interleaved per x chunk ----
        qmT_sb = persist.tile([DK, S], F16, tag="qmT")
        qT_sb = [persist.tile([PT, S], F16, tag=f"qT{i}", name=f"qT{i}")
                 for i in range(4)]
        kT_sb = [persist.tile([PT, S], F16, tag=f"kT{i}", name=f"kT{i}")
                 for i in range(4)]

        qmps = ph.tile([PT, S], F32, tag="ph", name="qmps")
        pq0 = pj.tile([PT, S], F32, tag="pj", name="projq0")
        for kc in range(NT):
            for nh in range(2):
                nhs = slice(nh * DHALF, (nh + 1) * DHALF)
                nc.tensor.matmul(qmps[0:DK, nhs], wm_sb[kc][:],
                                 x_tiles[kc][:, nhs],
                                 start=(kc == 0), stop=(kc == NT - 1))
            for nh in range(2):
                nhs = slice(nh * DHALF, (nh + 1) * DHALF)
                nc.tensor.matmul(pq0[:, nhs], wq_tiles[kc][:, 0:PT],
                                 x_tiles[kc][:, nhs],
                                 start=(kc == 0), stop=(kc == NT - 1))
        # prologue evacs on ACT (idle until the first exp)
        nc.scalar.activation(qmT_sb[:], qmps[0:DK, 0:S], AF.Identity,
                             bias=bm_sb[:], scale=1.0)
        nc.scalar.activation(qT_sb[0][:], pq0[:], AF.Identity,
                             bias=bq_sb[:, 0:1], scale=1.0)

        # ---- rel-bias W matmuls + wpad builds; wpad writes on the
        # scalar HWDGE queue, skew reads collected for the sync queue ----
        band_info = [(max(0, PT * (m - 1)), min(S, PT * (m + 2)))
                     for m in range(NT)]
        w0_sb = [persist.tile([PT, 1], F32, tag=f"w0_{m}", name=f"w0_{m}")
                 for m in range(NT)]
        w256_sb = [persist.tile([PT, 1], F32, tag=f"w256_{m}",
                                name=f"w256_{m}") for m in range(NT)]
        band_sb = [None] * NT
        wr_ins = [None] * NT

        def w_chain(m):
            jlo, jhi = band_info[m]
            ps = ph.tile([PT, S], F32, tag="ph", name=f"wps{m}")
            nc.tensor.matmul(ps[:, 0:NRELP], qmT_sb[:, m * PT:(m + 1) * PT],
                             tT_sb[:], start=True, stop=True)
            wp = wppool.tile([PT, WPADW], F16, tag="wpad", name=f"wpad{m}")
            nc.vector.tensor_copy(wp[:, MAXREL - 1:MAXREL - 1 + NREL],
                                  ps[:, 0:NREL])
            nc.vector.tensor_copy(w0_sb[m][:], ps[:, 0:1])
            nc.vector.tensor_copy(w256_sb[m][:], ps[:, NREL - 1:NREL])
            nc.vector.tensor_scalar_mul(wp[:, 0:MAXREL - 1], ones_sb[:],
                                        w0_sb[m][:])
            nc.vector.tensor_scalar_mul(wp[:, MAXREL - 1 + NREL:WPADW],
                                        ones_sb[:], w256_sb[m][:])
            wr_ins[m] = nc.scalar.dma_start(wpad_d[m * PT:(m + 1) * PT, :],
                                            wp[:])
            band_sb[m] = bpool.tile([PT, jhi - jlo], F16, tag=f"band{m}",
                                    name=f"band{m}")

        def band_read(m):
            jlo, jhi = band_info[m]
            srcap = bass.AP(wpad_d, PT * (WPADW - 1) * m + jlo + (WPADW // 2),
                            [[WPADW - 1, PT], [1, jhi - jlo]])
            ri = nc.sync.dma_start(band_sb[m][:], srcap)
            add_dep_helper(ri.ins, wr_ins[m].ins, reason="wpad DRAM RAW")

        for m in range(NT):
            w_chain(m)
        # sync-queue tail: two mask chunks, first band reads, rest of mask
        mask_load(0)
        mask_load(1)
        mask_term(0)
        mask_term(1)
        for m in range(4):
            band_read(m)
        for m in range(2, NT):
            mask_load(m)
            mask_term(m)
        for m in range(4, NT):
            band_read(m)

        # ---- t=0 k projection (k chunks land about now) ----
        pk0 = pj.tile([PT, S], F32, tag="pj", name="projk0")
        for kc in range(NT):
            for nh in range(2):
                nhs = slice(nh * DHALF, (nh + 1) * DHALF)
                nc.tensor.matmul(pk0[:, nhs], wk_tiles[kc][:, 0:PT],
                                 k_tiles[kc][:, nhs],
                                 start=(kc == 0), stop=(kc == NT - 1))
        nc.scalar.activation(kT_sb[0][:], pk0[:], AF.Identity,
                             bias=bk_sb[:, 0:1], scale=1.0)

        # ---- comb band chains (DVE); E = exp(comb) for E_MS (ACT) ----
        E_sb = {m: persist.tile([PT, S], F16, tag=f"E{m}", name=f"E{m}")
                for m in E_MS}

        def comb_chain(m):
            jlo, jhi = band_info[m]
            cb = comb_sb[m]
            nc.vector.tensor_add(cb[:, jlo:jhi], cb[:, jlo:jhi],
                                 band_sb[m][:])
            if jlo > 0:
                nc.vector.tensor_scalar_add(cb[:, 0:jlo], cb[:, 0:jlo],
                                            w0_sb[m][:])
            if jhi < S:
                nc.vector.tensor_scalar_add(cb[:, jhi:S], cb[:, jhi:S],
                                            w256_sb[m][:])

        comb_chain(0)
        comb_chain(1)

        # ---- projection op chunks for t>=1 ----
        def proj_ops(t, w_tiles, x_t, dst, bias_sb, nm):
            ps = [None]
            ops = []
            for kc in range(NT):
                for nh in range(2):
                    nhs = slice(nh * DHALF, (nh + 1) * DHALF)
                    def mm(kc=kc, nhs=nhs, first=(kc == 0 and nhs.start == 0)):
                        if first:
                            ps[0] = pj.tile([PT, S], F32, tag="pj",
                                            name=f"proj{nm}{t}")
                        nc.tensor.matmul(ps[0][:, nhs],
                                         w_tiles[kc][:, t * PT:(t + 1) * PT],
                                         x_t[kc][:, nhs],
                                         start=(kc == 0), stop=(kc == NT - 1))
                    ops.append(mm)

            def evac():
                nc.vector.tensor_scalar_add(dst[:], ps[0][:],
                                            bias_sb[:, t:t + 1])
            ops.append(evac)
            return ops

        # ---- main loop: 4 head pairs x 8 row tiles ----
        for t in range(4):
            pending = []
            if t < 3:
                pending = (proj_ops(t + 1, wq_tiles, x_tiles,
                                    qT_sb[t + 1], bq_sb, "q")
                           + proj_ops(t + 1, wk_tiles, k_tiles,
                                      kT_sb[t + 1], bk_sb, "k"))
            for m in range(NT):
                mb = slice(m * PT, (m + 1) * PT)
                is_id = m in ID_MS
                psA = ph.tile([PT, S], F32, tag="ph", name=f"psA_{t}_{m}")
                psB = ph.tile([PT, S], F32, tag="ph", name=f"psB_{t}_{m}")
                for hb, psx, tp in ((0, psA, (0, 0)), (1, psB, (64, 0))):
                    qsl = qT_sb[t][hb * DK:(hb + 1) * DK, mb]
                    for nh in range(2):
                        nhs = slice(nh * DHALF, (nh + 1) * DHALF)
                        nc.tensor.matmul(psx[:, nhs], qsl,
                                         kT_sb[t][hb * DK:(hb + 1) * DK, nhs],
                                         start=True, stop=not is_id,
                                         tile_position=tp)
                if is_id:
                    for psx in (psA, psB):
                        for nh in range(2):
                            nhs = slice(nh * DHALF, (nh + 1) * DHALF)
                            nc.tensor.matmul(psx[:, nhs], id_sb[:],
                                             comb_sb[m][:, nhs],
                                             start=False, stop=True)

                # deferred DVE comb chains + E exps, spread over t=0
                if t == 0 and m < 3:
                    comb_chain(2 * m + 2)
                    comb_chain(2 * m + 3)
                if t == 0 and 2 <= m < 6:
                    me = m + 2
                    nc.scalar.activation(E_sb[me][:], comb_sb[me][:], AF.Exp,
                                         bias=0.0, scale=1.0)

                e2 = epool.tile([PT, 2 * S], F16, tag="e", name=f"e{t}_{m}")
                S2 = spool.tile([PT, 2], F32, tag="s", name=f"s{t}_{m}")
                r2 = spool.tile([PT, 2], F32, tag="r", name=f"r{t}_{m}")
                o2 = opool.tile([PT, 2 * S], F16, tag="o", name=f"o{t}_{m}")
                srcs = []
                for hi, psx in ((0, psA), (1, psB)):
                    e = e2[:, hi * S:(hi + 1) * S]
                    sa = S2[:, hi:hi + 1]
                    if is_id:
                        nc.scalar.activation(e, psx[:], AF.Exp, bias=0.0,
                                             scale=1.0, accum_out=sa)
                        srcs.append(e)
                    else:
                        nc.scalar.activation(e, psx[:], AF.Exp, bias=0.0,
                                             scale=1.0)
                        mx = mpool.tile([PT, S], F16, tag="m",
                                        name=f"m{t}{hi}_{m}")
                        nc.vector.scalar_tensor_tensor(
                            mx[:], e, 1.0, E_sb[m][:], ALU.mult, ALU.mult,
                            accum_out=sa)
                        srcs.append(mx[:])
                nc.vector.reciprocal(r2[:], S2[:])
                for hi in range(2):
                    nc.vector.tensor_scalar_mul(o2[:, hi * S:(hi + 1) * S],
                                                srcs[hi], r2[:, hi:hi + 1])
                dst = bass.AP(out_d, (2 * t) * S * S + m * PT * S,
                              [[S, PT], [S * S, 2], [1, S]])
                nc.sync.dma_start(dst,
                                  o2[:].rearrange("p (h s) -> p h s", s=S))

                # feed pending projection matmuls to the PE in small chunks
                if pending and m >= 1:
                    nslots = NT - m
                    take = (len(pending) + nslots - 1) // nslots
                    for _ in range(min(take, 6)):
                        if pending:
                            pending.pop(0)()
            while pending:
                pending.pop(0)()

    nc.compile()
    return nc


def _get_nc():
    if "nc" not in _CACHE:
        _CACHE["nc"] = _build()
    return _CACHE["nc"]


def _prep_inputs(query, key, mask, Wq, bq, Wk, bk, rel_k_table):
    """Host-side sharding prep -> 8 per-core input dicts."""
    sc = 1.0 / np.sqrt(np.float32(DK))
    query = np.asarray(query, dtype=np.float32)
    key = np.asarray(key, dtype=np.float32)
    mask8 = np.ascontiguousarray(np.asarray(mask).astype(np.int8))
    Wq = np.asarray(Wq, dtype=np.float32)
    bq = np.asarray(bq, dtype=np.float32)
    Wk = np.asarray(Wk, dtype=np.float32)
    bk = np.asarray(bk, dtype=np.float32)
    T = np.asarray(rel_k_table, dtype=np.float32)

    WqTs = np.ascontiguousarray((Wq * sc).T)       # [D, D]
    WkT = np.ascontiguousarray(Wk.T)               # [D, D]
    bqs = bq * sc
    Wm16 = np.ascontiguousarray(((Wq.reshape(H, DK, D).mean(0) * sc).T).astype(np.float16))
    bm = (bq.reshape(H, DK).mean(0) * sc).reshape(DK, 1).astype(np.float32)
    tTc16 = np.zeros((DK, NRELP), np.float16)
    tTc16[:, :NREL] = T.T.astype(np.float16)       # [64, 260] zero-padded

    xT = [np.ascontiguousarray(query[b].T.astype(np.float16)) for b in range(B)]
    kT = [np.ascontiguousarray(key[b].T.astype(np.float16)) for b in range(B)]

    in_maps = []
    for c in range(NCORES):
        b, hh = divmod(c, 2)
        cols = slice(hh * DHALF, (hh + 1) * DHALF)
        in_maps.append(dict(
            xT=xT[b], kTx=kT[b], maskb=mask8[b],
            wqT=np.ascontiguousarray(WqTs[:, cols].astype(np.float16)),
            wkT=np.ascontiguousarray(WkT[:, cols].astype(np.float16)),
            bq4=np.ascontiguousarray(bqs[cols].reshape(4, PT).T),
            bk4=np.ascontiguousarray(bk[cols].reshape(4, PT).T),
            wmT=Wm16, bm1=bm, tT=tTc16,
        ))
    return in_maps


def run(inputs: dict, trace: bool = False):
    from concourse.bass_utils import run_bass_kernel_spmd

    nc = _get_nc()
    in_maps = _prep_inputs(**inputs)
    res = run_bass_kernel_spmd(nc, in_maps, core_ids=list(range(NCORES)),
                               trace=trace)
    out = np.empty((B, H, S, S), dtype=np.float32)
    for c in range(NCORES):
        b, hh = divmod(c, 2)
        out[b, hh * HPC:(hh + 1) * HPC] = res.results[c]["out"].astype(np.float32)
    return out, res


def kernel(**inputs) -> np.ndarray:
    out, _ = run(inputs)
    return out


# revision 35
# speedup vs baseline: 1.1067x; 1.0627x over previous
"""Bass/Trainium2 kernel for nn_EnhancedMultiHeadAttention (sparse_attention).

out[b,h,i,j] = softmax_j( (q_bh i . k_bh j) * sc + relbias[b,i,j] + mask_term[b,i,j] )
  q = query @ Wq.T + bq   (sc = 1/sqrt(64) folded into Wq/bq on host)
  relbias[b,i,j] = (mean_h q[b,h,i,:]) . rel_k_table[clip(j-i,-128,128)+128, :] * sc
  mask_term = 0 where mask==1 else -30000

Sharding: 8 cores = 4 batches x 2 head-halves (8 heads per core).

v8 design notes:
  - Every dma_start pays ~2us of serialized ring latency, so transfer
    COUNT matters as much as bytes: inputs are single/dual transfers
    spread over the three DGE rings (sync: x,k + band reads + half the
    outputs; scalar: wm,wq,wk + grouped wpad writes + other outputs;
    gpsimd: consts + mask single transfer).
  - Outputs fp16, batched two head-pairs per transfer (1MB, 4D access
    pattern), alternating between the sync and scalar rings.
  - comb[m] = (mask*30000-30000) + relbias band (DRAM skew trick, wpad
    writes/reads grouped 4 row tiles per transfer).
  - Row-tile classes: m 0-3 multiply-by-E (exp(s)*exp(comb), row sums
    via one 1x DVE scalar_tensor_tensor), m 4-7 identity-matmul comb on
    the PE with ACT accumulator row sums.  E tiles early (band DMA
    arrives just in time), identity tiles later (PE builds pipeline
    lead during the cheap E pairs, then absorbs the comb adds).
  - PSUM: per-head score tiles (ph, bufs=3) + separate projection pool
    (pj); projections emitted in small chunks between pairs.
"""

import numpy as np

B, S, D, H = 4, 1024, 1024, 16
DK = 64          # head dim
MAXREL = 128
NREL = 2 * MAXREL + 1          # 257
WPADW = 2 * MAXREL + NREL - 2  # 511 = 127 + 257 + 127
NRELP = 260     # rel matmul free dim padded for fp32r ISA restrictions
HPC = 8          # heads per core
DHALF = 512      # projected dims per core
NCORES = 8
PT = 128         # partition tile
NT = S // PT     # 8 row tiles

MASKV = 30000.0  # fp16-safe large negative bias for masked entries

E_MS = (0, 1, 2, 3)      # e*exp(comb), row sums via one DVE stt pass
ID_MS = (4, 5, 6, 7)     # identity-matmul comb + ACT accumulator sums

_CACHE = {}


def _build():
    from contextlib import ExitStack

    import concourse.bass as bass
    import concourse.mybir as mybir
    import concourse.tile as tile
    from concourse import bacc
    from concourse.tile import add_dep_helper

    F32 = mybir.dt.float32
    F16 = mybir.dt.float16
    I8 = mybir.dt.int8
    AF = mybir.ActivationFunctionType
    ALU = mybir.AluOpType

    nc = bacc.Bacc("TRN2", target_bir_lowering=False, debug=False)

    xT = nc.dram_tensor("xT", [D, S], F16, kind="ExternalInput")
    kTx = nc.dram_tensor("kTx", [D, S], F16, kind="ExternalInput")
    maskb = nc.dram_tensor("maskb", [S, S], I8, kind="ExternalInput")
    wqT = nc.dram_tensor("wqT", [D, DHALF], F16, kind="ExternalInput")
    wkT = nc.dram_tensor("wkT", [D, DHALF], F16, kind="ExternalInput")
    cst = nc.dram_tensor("cst", [PT, 9], F32, kind="ExternalInput")
    wmT = nc.dram_tensor("wmT", [D, DK], F16, kind="ExternalInput")
    tT = nc.dram_tensor("tT", [DK, NRELP], F16, kind="ExternalInput")
    out_d = nc.dram_tensor("out", [HPC, S, S], F16, kind="ExternalOutput")
    wpad_d = nc.dram_tensor("wpad_scratch", [S, WPADW], F16)
    ident_d = nc.inline_tensor(np.eye(PT, dtype=np.float16), "ident")

    with tile.TileContext(nc) as tc, ExitStack() as ctx:
        persist = ctx.enter_context(tc.tile_pool(name="persist", bufs=1))
        bpool = ctx.enter_context(tc.tile_pool(name="bpool", bufs=1))
        wppool = ctx.enter_context(tc.tile_pool(name="wppool", bufs=1))
        epool = ctx.enter_context(tc.tile_pool(name="epool", bufs=6))
        mpool = ctx.enter_context(tc.tile_pool(name="mpool", bufs=4))
        opool = ctx.enter_context(tc.tile_pool(name="opool", bufs=3))
        spool = ctx.enter_context(tc.tile_pool(name="spool", bufs=8))
        ph = ctx.enter_context(tc.tile_pool(name="ph", bufs=3, space="PSUM"))
        pj = ctx.enter_context(tc.tile_pool(name="pj", bufs=1, space="PSUM"))

        # trigger the ACT exp table load right away (off the critical path)
        tlw_sb = persist.tile([PT, 4], F32, tag="tlw")
        nc.vector.memset(tlw_sb[:], 0.0)
        nc.scalar.activation(tlw_sb[:], tlw_sb[:], AF.Exp, bias=0.0, scale=1.0)

        # ---- gpsimd ring: identity, rel table, packed f32 consts, mask ----
        id_sb = persist.tile([PT, PT], F16, tag="ident")
        nc.gpsimd.dma_start(id_sb[:], ident_d[:])
        tT_sb = persist.tile([DK, NRELP], F16, tag="tT")
        nc.gpsimd.dma_start(tT_sb[:], tT[:])
        cst_sb = persist.tile([PT, 9], F32, tag="cst")
        nc.gpsimd.dma_start(cst_sb[:], cst[:])
        bq_sb = cst_sb[:, 0:4]
        bk_sb = cst_sb[:, 4:8]
        bm_sb = cst_sb[0:DK, 8:9]
        mask_all = persist.tile([PT, NT * S], I8, tag="mask_all")
        nc.gpsimd.dma_start(
            mask_all[:].rearrange("p (c s) -> p c s", s=S),
            bass.AP(maskb, 0, [[S, PT], [PT * S, NT], [1, S]]))

        ones_sb = persist.tile([PT, MAXREL - 1], F32, tag="ones")
        nc.vector.memset(ones_sb[:], 1.0)

        # mask terms into comb tiles (gpsimd compute)
        comb_sb = [persist.tile([PT, S], F16, tag=f"comb{m}", name=f"comb{m}")
                   for m in range(NT)]
        for m in range(NT):
            nc.gpsimd.tensor_scalar(comb_sb[m][:],
                                    mask_all[:, m * S:(m + 1) * S],
                                    MASKV, -MASKV, ALU.mult, ALU.add)

        # ---- PE warmup while first DMAs land ----
        warm_sb = persist.tile([PT, DHALF], F16, tag="warm")
        nc.vector.memset(warm_sb[:], 0.0)
        wps = ph.tile([PT, S], F32, tag="ph", name="warmps")
        for i in range(6):
            nc.tensor.matmul(wps[:, 0:DHALF], id_sb[:], warm_sb[:],
                             start=True, stop=True)

        # ---- sync ring: x (2 parts), k (2 parts), band reads, outputs
        #      scalar ring: wm, wq, wk, wpad writes, outputs ----
        wm_all = persist.tile([PT, NT * DK], F16, tag="wm_all")
        nc.scalar.dma_start(
            wm_all[:].rearrange("p (c s) -> p c s", s=DK),
            bass.AP(wmT, 0, [[DK, PT], [PT * DK, NT], [1, DK]]))
        wm_sb = [wm_all[:, kc * DK:(kc + 1) * DK] for kc in range(NT)]

        x_all = persist.tile([PT, NT * S], F16, tag="x_all")
        for pi in range(2):
            nc.sync.dma_start(
                x_all[:, pi * 4 * S:(pi + 1) * 4 * S]
                .rearrange("p (c s) -> p c s", s=S),
                bass.AP(xT, pi * 4 * PT * S, [[S, PT], [PT * S, 4], [1, S]]))
        x_tiles = [x_all[:, kc * S:(kc + 1) * S] for kc in range(NT)]

        wq_all = persist.tile([PT, NT * DHALF], F16, tag="wq_all")
        nc.scalar.dma_start(
            wq_all[:].rearrange("p (c s) -> p c s", s=DHALF),
            bass.AP(wqT, 0, [[DHALF, PT], [PT * DHALF, NT], [1, DHALF]]))
        wq_tiles = [wq_all[:, kc * DHALF:(kc + 1) * DHALF] for kc in range(NT)]

        k_all = persist.tile([PT, NT * S], F16, tag="k_all")
        for pi in range(2):
            nc.sync.dma_start(
                k_all[:, pi * 4 * S:(pi + 1) * 4 * S]
                .rearrange("p (c s) -> p c s", s=S),
                bass.AP(kTx, pi * 4 * PT * S, [[S, PT], [PT * S, 4], [1, S]]))
        k_tiles = [k_all[:, kc * S:(kc + 1) * S] for kc in range(NT)]

        wk_all = persist.tile([PT, NT * DHALF], F16, tag="wk_all")
        nc.scalar.dma_start(
            wk_all[:].rearrange("p (c s) -> p c s", s=DHALF),
            bass.AP(wkT, 0, [[DHALF, PT], [PT * DHALF, NT], [1, DHALF]]))
        wk_tiles = [wk_all[:, kc * DHALF:(kc + 1) * DHALF] for kc in range(NT)]

        # ---- qm + t=0 q projection, interleaved per x part ----
        qmT_sb = persist.tile([DK, S], F16, tag="qmT")
        qT_sb = [persist.tile([PT, S], F16, tag=f"qT{i}", name=f"qT{i}")
                 for i in range(4)]
        kT_sb = [persist.tile([PT, S], F16, tag=f"kT{i}", name=f"kT{i}")
                 for i in range(4)]

        qmps = ph.tile([PT, S], F32, tag="ph", name="qmps")
        pq0 = pj.tile([PT, S], F32, tag="pj", name="projq0")
        for kc in range(NT):
            for nh in range(2):
                nhs = slice(nh * DHALF, (nh + 1) * DHALF)
                nc.tensor.matmul(qmps[0:DK, nhs], wm_sb[kc][:],
                                 x_tiles[kc][:, nhs],
                                 start=(kc == 0), stop=(kc == NT - 1))
            for nh in range(2):
                nhs = slice(nh * DHALF, (nh + 1) * DHALF)
                nc.tensor.matmul(pq0[:, nhs], wq_tiles[kc][:, 0:PT],
                                 x_tiles[kc][:, nhs],
                                 start=(kc == 0), stop=(kc == NT - 1))
        # prologue evacs on ACT (idle until the first exp)
        nc.scalar.activation(qmT_sb[:], qmps[0:DK, 0:S], AF.Identity,
                             bias=bm_sb, scale=1.0)
        nc.scalar.activation(qT_sb[0][:], pq0[:], AF.Identity,
                             bias=bq_sb[:, 0:1], scale=1.0)

        # ---- rel-bias W matmuls + wpad builds; grouped wpad writes on
        # the scalar ring, grouped skew reads on the sync ring ----
        band_info = [(max(0, PT * (m - 1)), min(S, PT * (m + 2)))
                     for m in range(NT)]
        w0_sb = [persist.tile([PT, 1], F32, tag=f"w0_{m}", name=f"w0_{m}")
                 for m in range(NT)]
        w256_sb = [persist.tile([PT, 1], F32, tag=f"w256_{m}",
                                name=f"w256_{m}") for m in range(NT)]
        wp_g = [wppool.tile([PT, 4 * WPADW], F16, tag=f"wpg{g}",
                            name=f"wpg{g}") for g in range(2)]
        band_g = [bpool.tile([PT, 4 * 384], F16, tag=f"bandg{g}",
                             name=f"bandg{g}") for g in range(2)]
        wr_ins = [None] * 2

        def w_chain(m):
            g, sl = divmod(m, 4)
            ps = ph.tile([PT, S], F32, tag="ph", name=f"wps{m}")
            nc.tensor.matmul(ps[:, 0:NRELP], qmT_sb[:, m * PT:(m + 1) * PT],
                             tT_sb[:], start=True, stop=True)
            wp = wp_g[g][:, sl * WPADW:(sl + 1) * WPADW]
            nc.vector.tensor_copy(wp[:, MAXREL - 1:MAXREL - 1 + NREL],
                                  ps[:, 0:NREL])
            nc.vector.tensor_copy(w0_sb[m][:], ps[:, 0:1])
            nc.vector.tensor_copy(w256_sb[m][:], ps[:, NREL - 1:NREL])
            nc.vector.tensor_scalar_mul(wp[:, 0:MAXREL - 1], ones_sb[:],
                                        w0_sb[m][:])
            nc.vector.tensor_scalar_mul(wp[:, MAXREL - 1 + NREL:WPADW],
                                        ones_sb[:], w256_sb[m][:])

        def wr_group(g):
            dst = bass.AP(wpad_d, g * 4 * PT * WPADW,
                          [[WPADW, PT], [PT * WPADW, 4], [1, WPADW]])
            wr_ins[g] = nc.scalar.dma_start(
                dst, wp_g[g][:].rearrange("p (c s) -> p c s", s=WPADW))

        def rd_group(g):
            # uniform skew read: for row-tile m, partition p reads
            # wpad[m*128+p, (jlo_u+jj)-(m*128+p)+255], jlo_u = 128*(m-1)
            m0 = g * 4
            base = PT * (WPADW - 1) * m0 + (PT * (m0 - 1)) + (WPADW // 2)
            grp_stride = PT * (WPADW - 1) + PT
            srcap = bass.AP(wpad_d, base,
                            [[WPADW - 1, PT], [grp_stride, 4], [1, 384]])
            ri = nc.sync.dma_start(
                band_g[g][:].rearrange("p (c s) -> p c s", s=384), srcap)
            add_dep_helper(ri.ins, wr_ins[g].ins, reason="wpad DRAM RAW")

        for m in range(4):
            w_chain(m)
        wr_group(0)
        for m in range(4, NT):
            w_chain(m)
        wr_group(1)
        rd_group(0)
        rd_group(1)

        def band_ap(m):
            g, sl = divmod(m, 4)
            jlo, jhi = band_info[m]
            off = jlo - PT * (m - 1)   # 128 for m=0, else 0
            return band_g[g][:, sl * 384 + off: sl * 384 + off + (jhi - jlo)]

        # ---- t=0 k projection ----
        pk0 = pj.tile([PT, S], F32, tag="pj", name="projk0")
        for kc in range(NT):
            for nh in range(2):
                nhs = slice(nh * DHALF, (nh + 1) * DHALF)
                nc.tensor.matmul(pk0[:, nhs], wk_tiles[kc][:, 0:PT],
                                 k_tiles[kc][:, nhs],
                                 start=(kc == 0), stop=(kc == NT - 1))
        nc.scalar.activation(kT_sb[0][:], pk0[:], AF.Identity,
                             bias=bk_sb[:, 0:1], scale=1.0)

        E_sb = {m: persist.tile([PT, S], F16, tag=f"E{m}", name=f"E{m}")
                for m in E_MS}

        def comb_chain(m):
            jlo, jhi = band_info[m]
            cb = comb_sb[m]
            nc.vector.tensor_add(cb[:, jlo:jhi], cb[:, jlo:jhi], band_ap(m))
            if jlo > 0:
                nc.vector.tensor_scalar_add(cb[:, 0:jlo], cb[:, 0:jlo],
                                            w0_sb[m][:])
            if jhi < S:
                nc.vector.tensor_scalar_add(cb[:, jhi:S], cb[:, jhi:S],
                                            w256_sb[m][:])

        for m in E_MS:
            comb_chain(m)

        # ---- projection op chunks for t>=1 ----
        def proj_ops(t, w_tiles, x_t, dst, bias_sb, nm):
            ps = [None]
            ops = []
            for kc in range(NT):
                for nh in range(2):
                    nhs = slice(nh * DHALF, (nh + 1) * DHALF)
                    def mm(kc=kc, nhs=nhs, first=(kc == 0 and nhs.start == 0)):
                        if first:
                            ps[0] = pj.tile([PT, S], F32, tag="pj",
                                            name=f"proj{nm}{t}")
                        nc.tensor.matmul(ps[0][:, nhs],
                                         w_tiles[kc][:, t * PT:(t + 1) * PT],
                                         x_t[kc][:, nhs],
                                         start=(kc == 0), stop=(kc == NT - 1))
                    ops.append(mm)

            def evac():
                nc.vector.tensor_scalar_add(dst[:], ps[0][:],
                                            bias_sb[:, t:t + 1])
            ops.append(evac)
            return ops

        # ---- main loop: 4 head pairs x 8 row tiles ----
        o4 = [None, None]
        for t in range(4):
            pending = []
            if t < 3:
                pending = (proj_ops(t + 1, wq_tiles, x_tiles,
                                    qT_sb[t + 1], bq_sb, "q")
                           + proj_ops(t + 1, wk_tiles, k_tiles,
                                      kT_sb[t + 1], bk_sb, "k"))
            for m in range(NT):
                mb = slice(m * PT, (m + 1) * PT)
                is_id = m in ID_MS
                psA = ph.tile([PT, S], F32, tag="ph", name=f"psA_{t}_{m}")
                psB = ph.tile([PT, S], F32, tag="ph", name=f"psB_{t}_{m}")
                for hb, psx, tp in ((0, psA, (0, 0)), (1, psB, (64, 0))):
                    qsl = qT_sb[t][hb * DK:(hb + 1) * DK, mb]
                    for nh in range(2):
                        nhs = slice(nh * DHALF, (nh + 1) * DHALF)
                        nc.tensor.matmul(psx[:, nhs], qsl,
                                         kT_sb[t][hb * DK:(hb + 1) * DK, nhs],
                                         start=True, stop=not is_id,
                                         tile_position=tp)
                if is_id:
                    for psx in (psA, psB):
                        for nh in range(2):
                            nhs = slice(nh * DHALF, (nh + 1) * DHALF)
                            nc.tensor.matmul(psx[:, nhs], id_sb[:],
                                             comb_sb[m][:, nhs],
                                             start=False, stop=True)

                # E tiles + id-class comb chains, spread through t=0;
                # E_sb[m] must be written before this pair's stt reads it
                if t == 0 and m in E_MS:
                    nc.scalar.activation(E_sb[m][:], comb_sb[m][:], AF.Exp,
                                         bias=0.0, scale=1.0)
                    comb_chain(m + 4)

                if m % 4 == 0:
                    o4[0] = opool.tile([PT, 4 * S], F16, tag="oA",
                                       name=f"oA{t}_{m}")
                    o4[1] = opool.tile([PT, 4 * S], F16, tag="oB",
                                       name=f"oB{t}_{m}")
                e2 = epool.tile([PT, 2 * S], F16, tag="e", name=f"e{t}_{m}")
                S2 = spool.tile([PT, 2], F32, tag="s", name=f"s{t}_{m}")
                r2 = spool.tile([PT, 2], F32, tag="r", name=f"r{t}_{m}")
                srcs = []
                for hi, psx in ((0, psA), (1, psB)):
                    e = e2[:, hi * S:(hi + 1) * S]
                    sa = S2[:, hi:hi + 1]
                    if is_id:
                        nc.scalar.activation(e, psx[:], AF.Exp, bias=0.0,
                                             scale=1.0, accum_out=sa)
                        srcs.append(e)
                    else:
                        nc.scalar.activation(e, psx[:], AF.Exp, bias=0.0,
                                             scale=1.0)
                        mx = mpool.tile([PT, S], F16, tag="m",
                                        name=f"m{t}{hi}_{m}")
                        nc.vector.scalar_tensor_tensor(
                            mx[:], e, 1.0, E_sb[m][:], ALU.mult, ALU.mult,
                            accum_out=sa)
                        srcs.append(mx[:])

                nc.vector.reciprocal(r2[:], S2[:])
                for hi in range(2):
                    nc.vector.tensor_scalar_mul(
                        o4[hi][:, (m % 4) * S:(m % 4) * S + S],
                        srcs[hi], r2[:, hi:hi + 1])
                if m % 4 == 3:
                    for hi, eng in ((0, nc.sync), (1, nc.scalar)):
                        dst = bass.AP(out_d,
                                      (2 * t + hi) * S * S + (m - 3) * PT * S,
                                      [[S, PT], [PT * S, 4], [1, S]])
                        eng.dma_start(
                            dst, o4[hi][:].rearrange("p (c s) -> p c s", s=S))

                # feed pending projection matmuls to the PE in small chunks
                if pending and m >= 1:
                    nslots = NT - m
                    take = (len(pending) + nslots - 1) // nslots
                    for _ in range(min(take, 6)):
                        if pending:
                            pending.pop(0)()
            while pending:
                pending.pop(0)()

    nc.compile()
    return nc


def _get_nc():
    if "nc" not in _CACHE:
        _CACHE["nc"] = _build()
    return _CACHE["nc"]


def _prep_inputs(query, key, mask, Wq, bq, Wk, bk, rel_k_table):
    """Host-side sharding prep -> 8 per-core input dicts."""
    sc = 1.0 / np.sqrt(np.float32(DK))
    query = np.asarray(query, dtype=np.float32)
    key = np.asarray(key, dtype=np.float32)
    mask8 = np.ascontiguousarray(np.asarray(mask).astype(np.int8))
    Wq = np.asarray(Wq, dtype=np.float32)
    bq = np.asarray(bq, dtype=np.float32)
    Wk = np.asarray(Wk, dtype=np.float32)
    bk = np.asarray(bk, dtype=np.float32)
    T = np.asarray(rel_k_table, dtype=np.float32)

    WqTs = np.ascontiguousarray((Wq * sc).T)       # [D, D]
    WkT = np.ascontiguousarray(Wk.T)               # [D, D]
    bqs = bq * sc
    Wm16 = np.ascontiguousarray(((Wq.reshape(H, DK, D).mean(0) * sc).T).astype(np.float16))
    bm = (bq.reshape(H, DK).mean(0) * sc).astype(np.float32)
    tTc16 = np.zeros((DK, NRELP), np.float16)
    tTc16[:, :NREL] = T.T.astype(np.float16)       # [64, 260] zero-padded

    xT = [np.ascontiguousarray(query[b].T.astype(np.float16)) for b in range(B)]
    kT = [np.ascontiguousarray(key[b].T.astype(np.float16)) for b in range(B)]

    in_maps = []
    for c in range(NCORES):
        b, hh = divmod(c, 2)
        cols = slice(hh * DHALF, (hh + 1) * DHALF)
        cstm = np.zeros((PT, 9), np.float32)
        cstm[:, 0:4] = bqs[cols].reshape(4, PT).T
        cstm[:, 4:8] = bk[cols].reshape(4, PT).T
        cstm[0:DK, 8] = bm
        in_maps.append(dict(
            xT=xT[b], kTx=kT[b], maskb=mask8[b],
            wqT=np.ascontiguousarray(WqTs[:, cols].astype(np.float16)),
            wkT=np.ascontiguousarray(WkT[:, cols].astype(np.float16)),
            cst=np.ascontiguousarray(cstm),
            wmT=Wm16, tT=tTc16,
        ))
    return in_maps


def run(inputs: dict, trace: bool = False):
    from concourse.bass_utils import run_bass_kernel_spmd

    nc = _get_nc()
    in_maps = _prep_inputs(**inputs)
    res = run_bass_kernel_spmd(nc, in_maps, core_ids=list(range(NCORES)),
                               trace=trace)
    out = np.empty((B, H, S, S), dtype=np.float32)
    for c in range(NCORES):
        b, hh = divmod(c, 2)
        out[b, hh * HPC:(hh + 1) * HPC] = res.results[c]["out"].astype(np.float32)
    return out, res


def kernel(**inputs) -> np.ndarray:
    out, _ = run(inputs)
    return out
